# revision 2
# baseline (speedup 1.0000x reference)
"""Trainium2 Bass kernel: AnaphoricityScorer (wl-coref pair FFNN scorer).

Data-parallel over the 512-row mention batch across 8 NeuronCores (64 rows
per core, 3200 pairs).  v3 splits the embedding by measured engine rates:

  - Table rows are packed [fp16 dims 0-511 | fp8 dims 512-1023] = 1536 B,
    gathered in ONE dma_gather(transpose=True) per block.  The 16-bit
    transpose granularity lands the fp16 half as normal K=128 chunks
    (dim 128c+p on partition p) and the fp8 half as byte-interleaved
    dim-pairs (512+2*(128c+p)+t) - exactly the fp8 DoubleRow K=256 layout.
  - Similarity: DVE fp16 multiply at the 2x rate (0.35 ns/elem) for the
    fp16 half, fp8 multiply (1.1 ns/elem) for the fp8 half.
  - Layer 1 PSUM accumulation per <=512-pair piece: 4 fp16 + 2 DoubleRow
    matmuls each for the b- and s-terms, one plain fp8 K=128 matmul for
    the stacked [W1p*32; ma*32] pw/a term.  All W1 scaled x32 so the fp8
    chunks sit in e4m3 range; the Lrelu activation applies 1/32.
  - Gather schedule: per queue one small 128-row block then one big block,
    so the first transfer drains before the second desc-gen finishes
    (racing them corrupts the second block - observed on HW).
  - GPSIMD ucode library (mlp) loaded via explicit load_library as the
    first GPSIMD instruction (page-in ~12.5 us runs behind const DMAs).
"""

import numpy as np
import ml_dtypes

N_MENTIONS = 10000
BATCH = 512
N_ANTS = 50
EMB = 1024
HALF = 512                      # dims 0-511 fp16, 512-1023 fp8
PW = 64
HID = 128
N_CORES = 8
R = BATCH // N_CORES            # 64 rows per core
NPAIR = R * N_ANTS              # 3200 pairs per core
NCH16 = HALF // 128             # 4 fp16 chunks (K=128)
NCH8 = HALF // 256              # 2 fp8 DoubleRow chunks (K=256)
ROWB = 2 * HALF + HALF          # 1536 bytes per packed table row
EPS = 1e-7
LEAKY = 0.01
WSCALE = 32.0
N_WARM = 48

F8 = ml_dtypes.float8_e4m3

# gather blocks: (size, queue) - small first per queue, then one big block
SIZES = [128, 128, 128, 128, 640, 640, 640, 768]
QUEUE = [0, 1, 2, 3, 0, 1, 2, 3]
OFFS = np.cumsum([0] + SIZES).tolist()
NB = len(SIZES)
assert OFFS[-1] == NPAIR and all(s % 128 == 0 for s in SIZES)

# compute pieces: (gather block, offset within block, length<=512)
PIECES = []
for _b in range(NB):
    _off = 0
    while _off < SIZES[_b]:
        _pl = min(512, SIZES[_b] - _off)
        PIECES.append((_b, _off, _pl))
        _off += _pl
NP_ = len(PIECES)
PBLKMAX = 512

# wcat8 (fp8): wb8 | ws8, each NCH8 chunks of [2, HID] cols
W8B0, W8S0 = 0, NCH8 * 2 * HID
WCAT8 = 2 * NCH8 * 2 * HID                       # 1024
# wcat16 (fp16): wb16 | ws16 (x32, 4 chunks x HID) | wa16 (x32, 8 x HID)
#                | mT16 (8 x R) | wout (1)
W16B0 = 0
W16S0 = NCH16 * HID
WA0 = 2 * NCH16 * HID
MT0 = WA0 + (EMB // 128) * HID
WO0 = MT0 + (EMB // 128) * R
WCAT16 = WO0 + 1
# pcat8 (fp8): moving stack [pw^T; e64] cols 0..NPAIR, stationary
# [W1p*32; ma*32] cols NPAIR..NPAIR+HID
PSTK0 = NPAIR
PCAT8 = NPAIR + HID
FB10, FBO0 = 0, 1
FCAT = 2

_CACHE = {}


def _build():
    if "nc" in _CACHE:
        return _CACHE["nc"]
    from concourse import bacc, mybir, library_config
    import concourse.tile as tile

    f8, f16, f32, i16 = (
        mybir.dt.float8e4, mybir.dt.float16, mybir.dt.float32, mybir.dt.int16
    )
    AF = mybir.ActivationFunctionType
    DRm = mybir.MatmulPerfMode.DoubleRow
    nc = bacc.Bacc(num_swdge_queues=4)

    def inp(name, shape, dtype):
        return nc.declare_dram_parameter(name, list(shape), dtype, isOutput=False)

    table = inp("table", [N_MENTIONS, ROWB], f8)
    idx = inp("idx", [128, NPAIR // 16], i16)
    wcat8 = inp("wcat8", [128, WCAT8], f8)
    wcat16 = inp("wcat16", [128, WCAT16], f16)
    at16 = inp("at16", [128, NCH16 * PBLKMAX], f16)
    at8 = inp("at8", [128, NCH8 * 2 * PBLKMAX], f8)
    pcat8 = inp("pcat8", [128, PCAT8], f8)
    fcat = inp("fcat", [128, FCAT], f32)
    rg = inp("rg", [128, NPAIR // 128], f32)
    out = nc.declare_dram_parameter("out", [128, NPAIR // 128], f32, isOutput=True)

    # Start the SWDGE ucode (mlp) page-in before the TileContext preamble
    # so the ~13us swap overlaps engine boot instead of following it.
    nc.gpsimd.load_library(library_config.mlp)

    with tile.TileContext(nc) as tc:
        with (
            tc.tile_pool(name="const", bufs=1) as cp,
            tc.tile_pool(name="bt", bufs=NB) as btp,
            tc.tile_pool(name="st", bufs=3) as stp,
            tc.tile_pool(name="hr", bufs=2) as hrp,
            tc.tile_pool(name="sm", bufs=2) as smp,
            tc.tile_pool(name="psH", bufs=4, space="PSUM") as psH,
            tc.tile_pool(name="psS", bufs=1, space="PSUM") as psS,
            tc.tile_pool(name="psM", bufs=1, space="PSUM") as psM,
        ):
            idx_sb = cp.tile([128, NPAIR // 16], i16, tag="idx")
            nc.sync.dma_start(out=idx_sb[:], in_=idx[:])

            warm_a = cp.tile([128, 128], f16, tag="warm_a")
            warm_b = cp.tile([128, 512], f16, tag="warm_b")
            nc.vector.memset(warm_a[:], 0)
            nc.vector.memset(warm_b[:], 0)
            warm_ps = psM.tile([128, 512], f32, tag="warm_ps")
            for _ in range(N_WARM):
                nc.tensor.matmul(
                    warm_ps[:], lhsT=warm_a[:], rhs=warm_b[:], start=True, stop=True
                )

            wcat8_sb = cp.tile([128, WCAT8], f8, tag="wcat8")
            nc.scalar.dma_start(out=wcat8_sb[:], in_=wcat8[:])
            wcat16_sb = cp.tile([128, WCAT16], f16, tag="wcat16")
            nc.scalar.dma_start(out=wcat16_sb[:], in_=wcat16[:])
            aT16_sb = cp.tile([128, NCH16 * PBLKMAX], f16, tag="aT16")
            nc.scalar.dma_start(out=aT16_sb[:], in_=at16[:])
            aT8_sb = cp.tile([128, NCH8 * 2 * PBLKMAX], f8, tag="aT8")
            nc.scalar.dma_start(out=aT8_sb[:], in_=at8[:])
            pcat8_sb = cp.tile([128, PCAT8], f8, tag="pcat8")
            nc.scalar.dma_start(out=pcat8_sb[:], in_=pcat8[:])
            fcat_sb = cp.tile([128, FCAT], f32, tag="fcat")
            nc.scalar.dma_start(out=fcat_sb[:], in_=fcat[:])
            rg128_sb = cp.tile([128, NPAIR // 128], f32, tag="rg")
            nc.scalar.dma_start(out=rg128_sb[:], in_=rg[:])

            wout_sb = wcat16_sb[:, WO0:WO0 + 1]
            b1_sb = fcat_sb[:, FB10:FB10 + 1]
            bout_sb = fcat_sb[:, FBO0:FBO0 + 1]

            def wb16_c(c):
                return wcat16_sb[:, W16B0 + c * HID:W16B0 + (c + 1) * HID]

            def ws16_c(c):
                return wcat16_sb[:, W16S0 + c * HID:W16S0 + (c + 1) * HID]

            def wb8_c(c):
                return wcat8_sb[:, W8B0 + c * 2 * HID:W8B0 + (c + 1) * 2 * HID] \
                    .rearrange("p (t m) -> p t m", t=2)

            def ws8_c(c):
                return wcat8_sb[:, W8S0 + c * 2 * HID:W8S0 + (c + 1) * 2 * HID] \
                    .rearrange("p (t m) -> p t m", t=2)

            def wa16_c(c):
                return wcat16_sb[:, WA0 + c * HID:WA0 + (c + 1) * HID]

            def mt16_c(c):
                return wcat16_sb[:, MT0 + c * R:MT0 + (c + 1) * R]

            bts = []
            for b in range(NB):
                L, o = SIZES[b], OFFS[b]
                bt = btp.tile([128, (ROWB // 256) * L], f16, tag="bt")
                nc.gpsimd.dma_gather(
                    out_ap=bt[:].bitcast(f8)
                    .rearrange("p (c n) -> p c n", c=ROWB // 128),
                    in_ap=table[:],
                    idxs_ap=idx_sb[:, o // 16:(o + L) // 16],
                    num_idxs=L,
                    num_idxs_reg=L,
                    elem_size=ROWB,
                    transpose=True,
                    queue_num=QUEUE[b],
                )
                bts.append(bt)

            scores_sb = cp.tile([128, NPAIR // 128], f32, tag="scores")
            sc_all = psS.tile([128, NPAIR // 128], f32, tag="sc_all")

            # ma*32 into pcat8's stationary stack (rows 64-127), fp8.
            ma_ps = psM.tile([128, HID], f32, tag="ma_ps")
            for c in range(EMB // 128):
                nc.tensor.matmul(
                    ma_ps[R:, :],
                    lhsT=mt16_c(c),
                    rhs=wa16_c(c),
                    start=(c == 0),
                    stop=(c == EMB // 128 - 1),
                )
            nc.scalar.activation(
                pcat8_sb[R:, PSTK0:PSTK0 + HID], ma_ps[R:, :], AF.Copy
            )

            LOOKAHEAD = 3
            hTs = {}
            hrs = {}

            def emit_l2(pj):
                hr_, go_, PL_ = hrs.pop(pj)
                for g in range(PL_ // 128):
                    nc.tensor.matmul(
                        sc_all[:, go_ // 128 + g:go_ // 128 + g + 1],
                        lhsT=hr_[:, g * 128:(g + 1) * 128],
                        rhs=wout_sb,
                        start=True,
                        stop=True,
                        skip_group_check=True,
                    )

            def emit_extras(pi):
                b, po, PL = PIECES[pi]
                go = OFFS[b] + po
                hT = psH.tile([128, PL], f32, tag="hT")
                hTs[pi] = hT
                nc.tensor.matmul(
                    hT[:],
                    lhsT=pcat8_sb[:, PSTK0:PSTK0 + HID],
                    rhs=pcat8_sb[:, go:go + PL],
                    start=True,
                    stop=False,
                    skip_group_check=True,
                )

            for pi in range(min(LOOKAHEAD, NP_)):
                emit_extras(pi)

            for pi in range(NP_):
                b, po, PL = PIECES[pi]
                L = SIZES[b]
                go = OFFS[b] + po
                bt = bts[b]
                # fp16 half: native f16 cols [0, 4L) -> chunks
                bt16 = bt[:, :4 * L] \
                    .rearrange("p (c n) -> p c n", c=NCH16)[:, :, po:po + PL]
                # fp8 half: f16 cols [4L, 6L) bitcast -> 2 chunks of (n t)
                bt8 = bt[:, 4 * L:].bitcast(f8) \
                    .rearrange("p (c n t) -> p c t n", c=NCH8, t=2)[:, :, :, po:po + PL]

                st16 = stp.tile([128, NCH16 * PL], f16, tag="st16")
                nc.vector.tensor_mul(
                    st16[:].rearrange("p (c n) -> p c n", c=NCH16),
                    bt16,
                    aT16_sb[:].rearrange("p (c n) -> p c n", c=NCH16)[:, :, :PL],
                )
                st8 = stp.tile([128, NCH8 * 2 * PL], f8, tag="st8")
                nc.vector.tensor_mul(
                    st8[:].rearrange("p (c n) -> p c n", c=NCH8),
                    bt[:, 4 * L:].bitcast(f8)
                    .rearrange("p (c n) -> p c n", c=NCH8)
                    [:, :, 2 * po:2 * (po + PL)],
                    aT8_sb[:].rearrange("p (c n) -> p c n", c=NCH8)[:, :, :2 * PL],
                )

                hT = hTs.pop(pi)
                for c in range(NCH16):
                    nc.tensor.matmul(
                        hT[:], lhsT=wb16_c(c), rhs=bt16[:, c],
                        start=False, stop=False, skip_group_check=True,
                    )
                for c in range(NCH8):
                    nc.tensor.matmul(
                        hT[:], lhsT=wb8_c(c), rhs=bt8[:, c],
                        start=False, stop=False, perf_mode=DRm,
                        skip_group_check=True,
                    )
                for c in range(NCH16):
                    nc.tensor.matmul(
                        hT[:], lhsT=ws16_c(c),
                        rhs=st16[:].rearrange("p (c n) -> p c n", c=NCH16)[:, c],
                        start=False, stop=False, skip_group_check=True,
                    )
                for c in range(NCH8):
                    nc.tensor.matmul(
                        hT[:], lhsT=ws8_c(c),
                        rhs=st8[:].rearrange("p (c n t) -> p c t n", c=NCH8, t=2)[:, c],
                        start=False, stop=(c == NCH8 - 1), perf_mode=DRm,
                        skip_group_check=True,
                    )
                if pi + LOOKAHEAD < NP_:
                    emit_extras(pi + LOOKAHEAD)

                hr = hrp.tile([128, PL], f16, tag="hr")
                nc.scalar.activation(
                    hr[:], hT[:], AF.Lrelu, bias=b1_sb, scale=1.0 / WSCALE,
                    alpha=LEAKY,
                )
                hrs[pi] = (hr, go, PL)
                # layer-2 deferred one piece so it never heads-of-line
                # blocks the next piece's matmuls behind the activation
                if pi > 0:
                    emit_l2(pi - 1)
                if pi == NP_ - 1:
                    emit_l2(pi)

            nc.vector.tensor_add(scores_sb[:], sc_all[:], rg128_sb[:])
            nc.sync.dma_start(out=out[:], in_=scores_sb[:])

    nc.compile()
    _CACHE["nc"] = nc
    return nc


def _host_shared(inputs):
    am = np.asarray(inputs["all_mentions"], np.float32)
    t16 = am[:, :HALF].astype(np.float16).view(np.uint8)     # [N, 1024] bytes
    t8 = am[:, HALF:].astype(F8).view(np.uint8)              # [N, 512] bytes
    table = np.concatenate([t16, t8], axis=1).view(F8)       # [N, 1536]

    W1 = np.asarray(inputs["W1"], np.float32)
    w1a, w1b, w1s, w1p = W1[:1024], W1[1024:2048], W1[2048:3072], W1[3072:]

    def f16_chunk(w, scale=1.0):
        n = w.shape[0] // 128
        return np.ascontiguousarray(
            (w * scale).reshape(n, 128, HID).transpose(1, 0, 2)
            .reshape(128, n * HID)
        ).astype(np.float16)

    def dr_chunk(w):
        # [512, 128] (dims 512-1023) -> [128, 2*2*128] fp8 DoubleRow chunks
        return np.ascontiguousarray(
            (w * WSCALE).reshape(NCH8, 128, 2, HID).transpose(1, 0, 2, 3)
            .reshape(128, NCH8 * 2 * HID)
        ).astype(F8)

    wcat16 = [
        f16_chunk(w1b[:HALF], WSCALE),    # wb16
        f16_chunk(w1s[:HALF], WSCALE),    # ws16
        f16_chunk(w1a, WSCALE),           # wa16 (full)
        # mT16 (full mentions transpose) is per-core; appended there
    ]
    return {
        "table": np.ascontiguousarray(table),
        "_wcat8": np.concatenate([dr_chunk(w1b[HALF:]), dr_chunk(w1s[HALF:])], axis=1),
        "_wcat16_shared": np.concatenate(wcat16, axis=1),
        "_w1p8": np.ascontiguousarray(w1p * WSCALE).astype(F8),
        "_e64": np.ascontiguousarray(
            np.tile(np.eye(R, dtype=np.float32), (1, N_ANTS))
        ).astype(F8),
        "_wout": np.asarray(inputs["W_out"], np.float32).astype(np.float16),
        "_b1c": np.asarray(inputs["b1"], np.float32).reshape(HID, 1),
        "_boutc": np.full(
            (128, 1), np.asarray(inputs["b_out"], np.float32).reshape(())
        ),
    }


def _host_core(inputs, shared, c):
    sl = slice(c * R, (c + 1) * R)
    m = np.asarray(inputs["mentions_batch"], np.float32)[sl]
    pw = np.asarray(inputs["pw_batch"], np.float32)[sl]
    idx = np.asarray(inputs["top_indices_batch"])[sl].astype(np.int64)
    rough = np.asarray(inputs["top_rough_scores_batch"], np.float32)[sl]

    idx_perm = idx.T.reshape(NPAIR).astype(np.int16)
    idx16 = np.concatenate(
        [
            np.tile(
                idx_perm[OFFS[b]:OFFS[b + 1]].reshape(SIZES[b] // 16, 16).T,
                (8, 1),
            )
            for b in range(NB)
        ],
        axis=1,
    )

    nf = EMB // 128
    mT16 = m.reshape(R, nf, 128).transpose(2, 1, 0).reshape(128, nf * R)
    # aT16: [128, c, n] = m[n % 64, 128c+p] over n in [0, 512)
    mth = m[:, :HALF].reshape(R, NCH16, 128).transpose(2, 1, 0)   # [128, 4, 64]
    at16 = np.tile(mth, (1, 1, PBLKMAX // R)).reshape(128, NCH16 * PBLKMAX)
    # wrong tiling order: need j-repeat of the WHOLE 64-run
    at16 = np.broadcast_to(
        mth[:, :, None, :], (128, NCH16, PBLKMAX // R, R)
    ).reshape(128, NCH16 * PBLKMAX)
    # aT8: [128, c, 2n+t] = m[n % 64, 512+256c+2p+t]
    mti = m[:, HALF:].reshape(R, NCH8, 128, 2).transpose(2, 1, 0, 3) \
        .reshape(128, NCH8, 2 * R)                                 # [128, 2, 128]
    at8 = np.broadcast_to(
        mti[:, :, None, :], (128, NCH8, PBLKMAX // R, 2 * R)
    ).reshape(128, NCH8 * 2 * PBLKMAX)

    pwT = pw.transpose(1, 0, 2).reshape(NPAIR, PW).T
    bo = np.asarray(inputs["b_out"], np.float32).reshape(())
    rough_pp = (rough + bo).T.reshape(NPAIR).reshape(NPAIR // 128, 128).T

    wcat16 = np.concatenate(
        [shared["_wcat16_shared"], mT16.astype(np.float16), shared["_wout"]],
        axis=1,
    )
    pcat_top = np.concatenate([pwT.astype(F8), shared["_w1p8"]], axis=1)
    pcat_bot = np.concatenate([shared["_e64"], np.zeros((R, HID), F8)], axis=1)
    pcat8 = np.concatenate([pcat_top, pcat_bot], axis=0)
    fcat = np.concatenate([shared["_b1c"], shared["_boutc"]], axis=1) \
        .astype(np.float32)

    return {
        "idx": np.ascontiguousarray(idx16),
        "wcat8": shared["_wcat8"],
        "wcat16": np.ascontiguousarray(wcat16),
        "at16": np.ascontiguousarray(at16).astype(np.float16),
        "at8": np.ascontiguousarray(at8).astype(F8),
        "pcat8": np.ascontiguousarray(pcat8),
        "fcat": np.ascontiguousarray(fcat),
        "rg": np.ascontiguousarray(rough_pp).astype(np.float32),
    }


def make_in_maps(inputs):
    shared = _host_shared(inputs)
    table = shared["table"]
    return [
        {"table": table, **_host_core(inputs, shared, c)} for c in range(N_CORES)
    ]


def assemble_output(inputs, results):
    scores = np.empty((BATCH, N_ANTS), np.float32)
    for c in range(N_CORES):
        out_flat = np.asarray(results[c]["out"], np.float32).T.reshape(NPAIR)
        scores[c * R:(c + 1) * R] = out_flat.reshape(N_ANTS, R).T
    dummy = np.full((BATCH, 1), EPS, np.float32)
    return np.concatenate([dummy, scores], axis=1)


def run(inputs, trace=False, **kwargs):
    from concourse.bass_utils import run_bass_kernel_spmd

    nc = _build()
    in_maps = make_in_maps(inputs)
    res = run_bass_kernel_spmd(
        nc, in_maps, core_ids=list(range(N_CORES)), trace=trace, **kwargs
    )
    return assemble_output(inputs, res.results), res


def _spot_check(inputs, out):
    """f32 recompute of one row per core; fp8-half noise is ~0.05 max,
    DMA corruption is O(1), so 0.3 separates them."""
    rows = np.array([c * R + ((13 * c) % R) for c in range(N_CORES)])
    am = np.asarray(inputs["all_mentions"], np.float32)
    m = np.asarray(inputs["mentions_batch"], np.float32)[rows]
    pw = np.asarray(inputs["pw_batch"], np.float32)[rows]
    idx = np.asarray(inputs["top_indices_batch"])[rows]
    rough = np.asarray(inputs["top_rough_scores_batch"], np.float32)[rows]
    W1 = np.asarray(inputs["W1"], np.float32)
    b1 = np.asarray(inputs["b1"], np.float32)
    Wo = np.asarray(inputs["W_out"], np.float32)
    bo = np.asarray(inputs["b_out"], np.float32)
    n = len(rows)
    a = np.broadcast_to(m[:, None, :], (n, N_ANTS, EMB))
    b = am[idx]
    pair = np.concatenate([a, b, a * b, pw], axis=2)
    h = pair @ W1 + b1
    h = np.where(h > 0, h, LEAKY * h)
    ref = rough + (h @ Wo)[..., 0] + bo[0]
    return np.abs(out[rows, 1:] - ref).max() < 0.3


def kernel(**inputs) -> np.ndarray:
    out = None
    for _ in range(3):
        out, _ = run(inputs, trace=False)
        if np.isfinite(out).all() and _spot_check(inputs, out):
            return out
    return out
